# revision 74
# baseline (speedup 1.0000x reference)
"""CARAFE + MSGConv Trainium2 kernel (8 NeuronCores, spatial x batch sharding).

out[c, i, j] = sum_{p,q} W[5p+q, i, j] * Xpad[c, i//2 + p - 2, j//2 + q - 2]
 (CARAFE taps live at source resolution; identical for both subpixel parities).

Per core: one batch element (core//4) and a 16-source-row block (core%4).

v2 design:
- Depthwise convs run on the TensorEngine as 25 accumulating diagonal
  matmuls per conv (lhsT = diag(w[:,t]) shipped from host), alternating
  two PSUM banks so weight loads pipeline with streaming.
- The 25-tap reassembly weights are scattered in PIXEL space (100 idx per
  row-pair instead of 400 in output space: no quadrant replication), then
  4 per-subpixel-plane PE transposes write an interleaved SBUF layout so
  each output block's matmul rhs is fully contiguous.
- Softmax runs per row-pair (exp -> strided reduce -> recip -> one
  broadcast tensor_tensor), pipelined with the scatter/transpose/matmul
  back end.
"""

import sys

sys.path.insert(0, "/opt/trn_rl_repo")

from contextlib import ExitStack

import ml_dtypes
import numpy as np

import concourse.bass as bass
import concourse.tile as tile
from concourse import bacc, library_config, mybir
from concourse.bass_utils import run_bass_kernel_spmd

BF16 = mybir.dt.bfloat16
F32 = mybir.dt.float32
I16 = mybir.dt.int16
AF = mybir.ActivationFunctionType
OP = mybir.AluOpType
nbf = ml_dtypes.bfloat16

C = 128
H = W = 64
NCORES = 8
XR = 24          # X shard rows (16 + 4 halo each side)
XW = 68          # padded width for dw slabs only
NEG = -30.0      # additive pre-activation mask; SiLU(-30) ~= -2.8e-12


# ======================================================================
# host-side parameter prep
# ======================================================================

def _fold_1x1(w, s):
    return (w[:, :, 0, 0] * s[:, None]).T.copy()


def _dw_taps(w, s, k):
    ch = w.shape[0]
    out = np.zeros((ch, 25), np.float32)
    off = (5 - k) // 2
    for ty in range(k):
        for tx in range(k):
            out[:, 5 * (ty + off) + (tx + off)] = w[:, 0, ty, tx] * s
    return out


def _host_consts(inputs):
    d = {}
    w_cv1 = _fold_1x1(inputs["comp_cv1_w"], inputs["comp_cv1_s"])
    b_cv1 = inputs["comp_cv1_b"]
    w3 = _dw_taps(inputs["comp_dw3_w"], inputs["comp_dw3_s"], 3)
    w5 = _dw_taps(inputs["comp_dw5_w"], inputs["comp_dw5_s"], 5)
    w_dwp = np.tile(np.concatenate([w3, w5], 0), (4, 1))      # [128, 25]
    b_dwp = np.tile(
        np.concatenate([inputs["comp_dw3_b"], inputs["comp_dw5_b"]]), 4
    )
    w_px = _fold_1x1(inputs["comp_px_w"], inputs["comp_px_s"])
    b_px = inputs["comp_px_b"]
    we = _fold_1x1(inputs["enc_cv1_w"], inputs["enc_cv1_s"])
    w_ecv1 = np.concatenate([we, np.ones((1, 50), np.float32)], 0)
    b_ecv1 = inputs["enc_cv1_b"]
    e3 = _dw_taps(inputs["enc_dw3_w"], inputs["enc_dw3_s"], 3)
    e5 = _dw_taps(inputs["enc_dw5_w"], inputs["enc_dw5_s"], 5)
    w_edw1 = np.concatenate([e3, e5], 0)                      # [50, 25]
    w_edwp = np.zeros((128, 25), np.float32)
    w_edwp[0:50] = w_edw1
    w_edwp[64:114] = w_edw1
    b_edw1 = np.concatenate([inputs["enc_dw3_b"], inputs["enc_dw5_b"]])
    b_edwp = np.zeros(128, np.float32)
    b_edwp[0:50] = b_edw1
    b_edwp[64:114] = b_edw1
    wpx = _fold_1x1(inputs["enc_px_w"], inputs["enc_px_s"])
    w_epx = np.concatenate([wpx, inputs["enc_px_b"].reshape(1, 100)], 0)

    # packa bf16 [128, 712]:
    #   0:128    w_cv1 replicated 4x along M (out partition = 32g+ch)
    #   128:256  w_ecv1 replicated (out partition = 64G+c, 32-aligned)
    #   256:356  w_epx
    #   356:484  identity
    #   484:740  w_px group replicas: K=64 blocks, inactive half zeroed
    pa = np.zeros((128, 740), np.float32)
    for g in range(4):
        pa[0:128, 32 * g : 32 * g + 32] = w_cv1
    pa[0:64, 128:178] = w_ecv1[0:64, :]
    pa[0:64, 192:242] = w_ecv1[0:64, :]
    pa[64, 128:178] = 1.0
    pa[64, 192:242] = 1.0
    pa[0:101, 256:356] = w_epx
    pa[0:128, 356:484] = np.eye(128)
    for half in range(2):
        for h in range(2):
            rows = slice(64 * half + 32 * h, 64 * half + 32 * h + 32)
            pa[rows, 484 + 64 * h : 548 + 64 * h] = w_px[0:32]
            pa[rows, 612 + 64 * h : 676 + 64 * h] = w_px[32:64]
    d["packa"] = pa.astype(nbf)
    # packb f32 [128, 5]: biases (cv1/ecv1 tiled to match replicated M)
    pb = np.zeros((128, 5), np.float32)
    pb[:, 0] = b_dwp
    pb[:, 1] = b_edwp
    pb[:, 2] = np.tile(b_cv1, 4)
    pb[0:64, 3] = b_px
    pb[0:50, 4] = b_ecv1
    pb[64:114, 4] = b_ecv1
    d["packb"] = pb

    # depthwise tap weights (diag matrices are built on-device by scatter);
    # chunk layout: 12 taps + 14 taps (num_idxs must be even; the 14th
    # duplicates tap 24, writing the same diag slot twice)
    wt = np.zeros((128, 52), np.float32)
    wt[:, 0:12] = w_dwp[:, 0:12]
    wt[:, 12:25] = w_dwp[:, 12:25]
    wt[:, 25] = w_dwp[:, 24]
    wt[:, 26:38] = w_edwp[:, 0:12]
    wt[:, 38:51] = w_edwp[:, 12:25]
    wt[:, 51] = w_edwp[:, 24]
    d["wtap"] = wt.astype(nbf)
    # scatter indices for the diag build: col j -> diag block j, own row
    dg = np.zeros((128, 14), np.int16)
    for p in range(128):
        for j in range(14):
            dg[p, j] = 128 * min(j, 12) + p
    d["dgix"] = dg

    d["ones1"] = np.ones((1, 128), nbf)
    d["erow1"] = np.ones((1, 16 * W), nbf)

    # sidx [128, 100] int16: pixel-space scatter, pix = 64*yl + x
    # entry e = 4k+s -> 120*s + 20*(yl+p) + (x%16 + q), k = 5p+q
    si = np.zeros((128, 100), np.int16)
    for pix in range(128):
        yl, x = divmod(pix, 64)
        xl = x % 16
        for k in range(25):
            p, q = divmod(k, 5)
            for s in range(4):
                si[pix, 4 * k + s] = 120 * s + 20 * (yl + p) + (xl + q)
    d["sidx"] = si
    return d


def _host_shard(X, core):
    b, ri = divmod(core, 4)
    r0 = 16 * ri - 4
    xs = np.zeros((C, XR, W), np.float32)
    lo, hi = max(0, r0), min(H, r0 + XR)
    xs[:, lo - r0 : hi - r0, :] = X[b, :, lo:hi, :]
    mrow = np.zeros((1, XR, W), np.float32)
    for r in range(XR):
        if not (0 <= r0 + r < H):
            mrow[0, r, :] = NEG
    emask = np.zeros((1, 20, W), np.float32)
    for r in range(20):
        if not (0 <= (16 * ri - 2) + r < H):
            emask[0, r, :] = NEG
    xsb = xs.astype(nbf)
    # pre-transposed X slabs, one [120, 128] per block (column-padded)
    xsp = np.zeros((C, XR, XW), nbf)
    xsp[:, :, 2 : 2 + W] = xsb
    xt = np.zeros((120, 32 * 128), nbf)
    for B in range(32):
        t, jb = divmod(B, 4)
        slab = xsp[:, 2 * t + 2 : 2 * t + 8, 16 * jb : 16 * jb + 20]
        xt[:, 128 * B : 128 * B + 128] = slab.reshape(C, 120).T
    return (
        xsb.reshape(C, XR * W),
        mrow.reshape(1, XR * W).astype(nbf),
        emask.reshape(1, 20 * W).astype(nbf),
        xt,
    )


# ======================================================================
# device kernel
# ======================================================================

def build_kernel():
    nc = bacc.Bacc(
        "TRN2",
        target_bir_lowering=False,
        debug=False,
        enable_asserts=False,
        num_devices=NCORES,
    )

    def din(name, shape, dt):
        return nc.dram_tensor(name, list(shape), dt, kind="ExternalInput").ap()

    x_d = din("x", (128, XR * W), BF16)
    xt_d = din("xt", (120, 32 * 128), BF16)
    mrow_d = din("mrow", (1, XR * W), BF16)
    emask_d = din("emask", (1, 20 * W), BF16)
    erow1_d = din("erow1", (1, 16 * W), BF16)
    ones1_d = din("ones1", (1, 128), BF16)
    packa_d = din("packa", (128, 740), BF16)
    packb_d = din("packb", (128, 5), F32)
    wtap_d = din("wtap", (128, 52), BF16)
    dgix_d = din("dgix", (128, 14), I16)
    sidx_d = din("sidx", (128, 100), I16)
    out_d = nc.dram_tensor("out", [128, 32 * 128], F32, kind="ExternalOutput").ap()
    out3 = out_d.rearrange("c (r j) -> c r j", j=128)

    with tile.TileContext(nc) as tc, ExitStack() as ctx:
        cpool = ctx.enter_context(tc.tile_pool(name="consts", bufs=1))
        work = ctx.enter_context(tc.tile_pool(name="work", bufs=1))
        spool = ctx.enter_context(tc.tile_pool(name="stage", bufs=3))
        psA_cm = tc.tile_pool(name="psA", bufs=2, space="PSUM")
        psA = psA_cm.__enter__()
        psDW_cm = tc.tile_pool(name="psDW", bufs=1, space="PSUM")
        psDW = psDW_cm.__enter__()

        nc.gpsimd.load_library(library_config.local_scatter)

        def cload(ap_d, shape, dt, eng=None):
            t = cpool.tile(list(shape), dt, tag=ap_d.tensor.name)
            (eng or nc.sync).dma_start(t[:], ap_d)
            return t

        # sync queue: cv1 weights first, then the X chunks, then the rest
        packa = cpool.tile([128, 740], BF16, tag="packa")
        nc.sync.dma_start(packa[:, 0:128], packa_d[:, 0:128])
        packb = cload(packb_d, (128, 5), F32, eng=nc.scalar)
        mrow = cload(mrow_d, (1, XR * W), BF16, eng=nc.scalar)
        ones1 = cload(ones1_d, (1, 128), BF16, eng=nc.scalar)
        w_cv1 = packa[0:128, 0:128]
        w_ecv1 = packa[0:65, 128:256]
        w_epx = packa[0:101, 256:356]
        ident = packa[0:128, 356:484]
        b_dwp = packb[0:128, 0:1]
        b_edwp = packb[0:128, 1:2]
        b_px = packb[0:64, 3:4]
        xb = cpool.tile([128, XR * W], BF16, tag="x")
        wtap = cload(wtap_d, (128, 52), BF16, eng=nc.gpsimd)
        dgix = cload(dgix_d, (128, 14), I16, eng=nc.gpsimd)
        for ch in range(3):
            (nc.sync if ch != 1 else nc.scalar).dma_start(
                xb[:, 8 * W * ch : 8 * W * (ch + 1)],
                x_d[:, 8 * W * ch : 8 * W * (ch + 1)],
            )
        nc.sync.dma_start(packa[:, 128:740], packa_d[:, 128:740])

        # warmup: trigger the local_scatter ucode library load early
        warm = work.tile([16, 16], BF16)
        nc.gpsimd.local_scatter(
            warm[:], packa[0:16, 0:2], dgix[:][0:16, 0:2],
            channels=16, num_elems=16, num_idxs=2,
        )

        # build the depthwise diag matrices on-device (scatter zero-fills)
        diagc = cpool.tile([128, 3200], BF16, tag="diagc")
        diage = cpool.tile([128, 3200], BF16, tag="diage")
        nc.gpsimd.local_scatter(
            diagc[:, 0:1536], wtap[:, 0:12], dgix[:, 0:12],
            channels=128, num_elems=1536, num_idxs=12,
        )
        nc.gpsimd.local_scatter(
            diagc[:, 1536:3200], wtap[:, 12:26], dgix[:, 0:14],
            channels=128, num_elems=1664, num_idxs=14,
        )
        nc.gpsimd.local_scatter(
            diage[:, 0:1536], wtap[:, 26:38], dgix[:, 0:12],
            channels=128, num_elems=1536, num_idxs=12,
        )
        nc.gpsimd.local_scatter(
            diage[:, 1536:3200], wtap[:, 38:52], dgix[:, 0:14],
            channels=128, num_elems=1664, num_idxs=14,
        )
        sidx = cload(sidx_d, (128, 100), I16, eng=nc.gpsimd)
        xt = cpool.tile([120, 32 * 128], BF16, tag="xt")
        for ch in range(2):
            nc.gpsimd.dma_start(
                xt[:, 2048 * ch : 2048 * (ch + 1)],
                xt_d[:, 2048 * ch : 2048 * (ch + 1)],
            )

        # persistent working tensors
        enc_in = work.tile([65, 20 * W], BF16)     # px out + mask row
        enc_cat = work.tile([101, 16 * W], BF16)   # enc x1/x2 + ones row
        x1p = work.tile([128, 9 * XW + 8], BF16)   # packed x1 (68-pitch)
        e1p = work.tile([128, 12 * XW + 8], BF16)  # packed enc x1 (68-pitch)
        ET = work.tile([128, 800], F32)            # enc px logits
        expb = work.tile([128, 800], BF16)         # exp values [t][4k+s]
        S = work.tile([128, 32], F32)
        R = work.tile([128, 32], F32)
        wcats = work.tile([128, 800], BF16)        # softmaxed weights
        b4x = work.tile([128, 8 * 480], BF16)      # pix-space bands
        x2p = work.tile([128, 340], BF16)
        e2p = work.tile([128, 544], BF16)

        xb3 = xb[:].rearrange("p (r c) -> p r c", c=W)
        enc_cat3 = enc_cat[:].rearrange("p (r c) -> p r c", c=W)
        x1p3 = x1p[:, 0 : 9 * XW].rearrange("p (r c) -> p r c", c=XW)
        e1p3 = e1p[:, 0 : 12 * XW].rearrange("p (r c) -> p r c", c=XW)
        ET3 = ET[:].rearrange("p (t e) -> p t e", e=100)
        Rv = R[:].rearrange("p (t s) -> p t s", s=4)
        Sv = S[:].rearrange("p (t s) -> p t s", s=4)

        # zero the dw-slab pad columns (cols 0:2 and 66:68)
        nc.vector.memset(x1p[:, 9 * XW : 9 * XW + 8], 0.0)
        nc.vector.memset(x1p3[:, :, 0:2], 0.0)
        nc.vector.memset(x1p3[:, :, 66:68], 0.0)
        nc.vector.memset(e1p[:], 0.0)
        nc.sync.dma_start(enc_cat[100:101, :], erow1_d)
        nc.sync.dma_start(enc_in[64:65, :], emask_d)

        # ---- comp cv1: 1x1 conv 128->32, M replicated 4x so the SiLU
        # writes the packed 68-pitch x1p slabs directly from PSUM
        # (ci chunk of 8 x-rows) -> per-group (psum_row0, nrows, local_row0)
        CV1_W = (
            ((0, 0, 8, 0), (1, 5, 3, 0)),
            ((0, 0, 1, 8), (1, 0, 6, 3), (2, 2, 6, 0), (3, 7, 1, 0)),
            ((2, 0, 3, 6), (3, 0, 8, 1)),
        )
        for ci in range(3):
            ps = psA.tile([128, 512], F32, tag="convps")
            nc.tensor.matmul(
                ps[:], w_cv1, xb[:, 512 * ci : 512 * (ci + 1)],
                start=True, stop=False,
            )
            nc.tensor.matmul(
                ps[:], ones1[:], mrow[:, 512 * ci : 512 * (ci + 1)],
                start=False, stop=True,
            )
            ps3 = ps[:].rearrange("p (r c) -> p r c", c=W)
            for g, pr, nr, lr in CV1_W[ci]:
                nc.scalar.activation(
                    x1p3[32 * g : 32 * g + 32, lr : lr + nr, 2 : 2 + W],
                    ps3[32 * g : 32 * g + 32, pr : pr + nr, :],
                    AF.Silu, bias=packb[32 * g : 32 * g + 32, 2:3],
                )

        # ---- comp dw3/dw5: 25 diag matmuls, parity-split PSUM banks
        # even taps accumulate in bank A, odd in bank B (pipelined LDW)
        ps_c0 = psDW.tile([128, 340], F32, tag="dwc0")
        ps_c1 = psDW.tile([128, 340], F32, tag="dwc1")
        for t in range(25):
            ty, tx = divmod(t, 5)
            off = ty * XW + tx
            nc.tensor.matmul((ps_c0 if t % 2 == 0 else ps_c1)[:],
                             diagc[:, 128 * t : 128 * t + 128],
                             x1p[:, off : off + 340],
                             start=(t < 2), stop=(t >= 23))
        tmpb = work.tile([128, 340], BF16)
        accd = work.tile([128, 340], BF16)
        nc.scalar.copy(tmpb[:], ps_c1[:])
        nc.vector.tensor_tensor(accd[:], ps_c0[:], tmpb[:], OP.add)
        nc.scalar.activation(x2p[:], accd[:], AF.Silu, bias=b_dwp)

        # ---- comp px: 1x1 conv 64->64 (+ SiLU), reading the packed
        # group layouts directly (x1 from x1p rows 2:7, x2 from x2p)
        x2p3 = x2p[:].rearrange("p (r c) -> p r c", c=XW)
        for g in range(4):
            b, h = 64 * (g // 2), g % 2
            ps = psA.tile([64, 512], F32, tag="convps")
            nc.tensor.matmul(
                ps[:, 0:320], packa[b : b + 64, 484 + 64 * h : 548 + 64 * h],
                x1p3[b : b + 64, 2:7, 2 : 2 + W],
                start=True, stop=False,
            )
            nc.tensor.matmul(
                ps[:, 0:320], packa[b : b + 64, 612 + 64 * h : 676 + 64 * h],
                x2p3[b : b + 64, 0:5, 0:W],
                start=False, stop=True,
            )
            nc.scalar.activation(
                enc_in[0:64, 320 * g : 320 * (g + 1)], ps[:, 0:320],
                AF.Silu, bias=b_px,
            )

        # ---- enc cv1: 1x1 conv 64->50, M replicated 2x so the SiLU
        # writes the packed e1p slabs and enc_cat x1 rows from PSUM
        ECV1_E1P = (
            ((0, 0, 8, 0),),
            ((0, 0, 4, 8), (1, 0, 8, 0)),
            ((1, 0, 4, 8),),
        )
        for ci, (r0, nr) in enumerate(((0, 8), (8, 8), (16, 4))):
            ps = psA.tile([128, 512], F32, tag="convps")
            nc.tensor.matmul(
                ps[:, : nr * W], w_ecv1,
                enc_in[0:65, r0 * W : (r0 + nr) * W],
                start=True, stop=True,
            )
            ps3 = ps[:].rearrange("p (r c) -> p r c", c=W)
            for G, pr, nrr, lr in ECV1_E1P[ci]:
                nc.scalar.activation(
                    e1p3[64 * G : 64 * G + 50, lr : lr + nrr, 2 : 2 + W],
                    ps3[64 * G : 64 * G + 50, pr : pr + nrr, :],
                    AF.Silu, bias=packb[64 * G : 64 * G + 50, 4:5],
                )
        # enc_cat x1 rows duplicate e1p data; copy via DMA, hidden
        # under the enc dw phase (enc_cat is needed ~10us later)
        nc.sync.dma_start(enc_cat3[0:50, 0:10, :],
                          e1p3[0:50, 2:12, 2 : 2 + W])
        nc.gpsimd.dma_start(enc_cat3[0:50, 10:16, :],
                            e1p3[64:114, 4:10, 2 : 2 + W])

        # ---- enc dw3/dw5: 25 diag matmuls, 2 PSUM chunks of 272
        # 2-bank rotation (chunk0/chunk1): LDW pipelines with streaming
        ps_e0 = psDW.tile([128, 272], F32, tag="dwe0")
        ps_e1 = psDW.tile([128, 272], F32, tag="dwe1")
        for t in range(25):
            ty, tx = divmod(t, 5)
            off = ty * XW + tx
            lhsT = diage[:, 128 * t : 128 * t + 128]
            nc.tensor.matmul(ps_e0[:], lhsT, e1p[0:128, off : off + 272],
                             start=(t == 0), stop=(t == 24))
            nc.tensor.matmul(ps_e1[:], lhsT, e1p[0:128, off + 272 : off + 544],
                             start=(t == 0), stop=(t == 24))
        nc.scalar.activation(e2p[:, 0:272], ps_e0[:], AF.Silu, bias=b_edwp)
        nc.scalar.activation(e2p[:, 272:544], ps_e1[:], AF.Silu, bias=b_edwp)
        for g in range(2):
            (nc.sync if g == 0 else nc.scalar).dma_start(
                enc_cat3[50:100, 8 * g : 8 * g + 8, :],
                e2p[64 * g : 64 * g + 50, :].rearrange(
                    "p (r c) -> p r c", c=XW
                )[:, 0:8, 0:W],
            )

        psDW_cm.__exit__(None, None, None)
        psA_cm.__exit__(None, None, None)
        psE_cm = tc.tile_pool(name="psE", bufs=2, space="PSUM")
        psE = psE_cm.__enter__()

        # ---- enc px (transposed output: M = 128 pixels per row-pair)
        for t in range(8):
            ps = psE.tile([128, 100], F32, tag="encpx")
            nc.tensor.matmul(
                ps[:], enc_cat[0:101, 128 * t : 128 * t + 128],
                w_epx, start=True, stop=True,
            )
            nc.scalar.activation(ET[:, 100 * t : 100 * t + 100], ps[:], AF.Silu)

        psE_cm.__exit__(None, None, None)
        psT = ctx.enter_context(tc.tile_pool(name="psT", bufs=3, space="PSUM"))
        psO = ctx.enter_context(tc.tile_pool(name="psO", bufs=3, space="PSUM"))

        # ---- per row-pair: softmax -> scatter -> transpose -> matmul
        # (exp/reduce/recip batched per t-pair to amortize op overhead)
        for t in range(8):
            te = expb[:, 100 * t : 100 * t + 100]
            tw = wcats[:, 100 * t : 100 * t + 100]
            if t < 2:
                nc.scalar.activation(te, ET[:, 100 * t : 100 * t + 100],
                                     AF.Exp)
                nc.vector.tensor_reduce(
                    Sv[:, t], te.rearrange("p (k s) -> p s k", s=4),
                    mybir.AxisListType.X, OP.add,
                )
                nc.vector.reciprocal(R[:, 4 * t : 4 * t + 4],
                                     S[:, 4 * t : 4 * t + 4])
            elif t % 2 == 0:
                nc.scalar.activation(expb[:, 100 * t : 100 * t + 200],
                                     ET[:, 100 * t : 100 * t + 200], AF.Exp)
                for u in (t, t + 1):
                    nc.vector.tensor_reduce(
                        Sv[:, u],
                        expb[:, 100 * u : 100 * u + 100].rearrange(
                            "p (k s) -> p s k", s=4),
                        mybir.AxisListType.X, OP.add,
                    )
                nc.vector.reciprocal(R[:, 4 * t : 4 * t + 8],
                                     S[:, 4 * t : 4 * t + 8])
            # scale: exp * R, R broadcast over k
            rb = Rv[:, t].unsqueeze(1).broadcast_to([128, 25, 4])
            nc.vector.tensor_tensor(
                tw.rearrange("p (k s) -> p k s", s=4),
                te.rearrange("p (k s) -> p k s", s=4), rb, OP.mult,
            )
            # pixel-space band scatter
            bx = b4x[:, 480 * t : 480 * t + 480]
            nc.gpsimd.local_scatter(
                bx, tw, sidx[:], channels=128, num_elems=480, num_idxs=100,
            )
            # 4 plane transposes -> interleaved b4all
            b4all = spool.tile([120, 512], BF16, tag="b4all")
            bview = b4all[:].rearrange(
                "p (jb yl dy xl dx) -> p dy dx yl jb xl",
                jb=4, yl=2, dy=2, xl=16, dx=2,
            )
            for s in range(4):
                pst = psT.tile([120, 128], BF16, tag="tr")
                nc.tensor.transpose(pst[:], bx[:, 120 * s : 120 * s + 120],
                                    ident)
                src = pst[:].rearrange("p (yl jb xl) -> p yl jb xl",
                                       yl=2, jb=4)
                dst = bview[:, s // 2, s % 2]
                if s % 2 == 0:
                    nc.vector.tensor_copy(dst, src)
                else:
                    nc.scalar.copy(dst, src)
            # 4 output matmuls into one PSUM bank, written pre-interleaved
            # (block jb's (r, j) columns land at psum col r*128 + 32*jb + j)
            po = psO.tile([128, 512], F32, tag="out")
            po3 = po[:].rearrange("c (r j) -> c r j", j=128)
            for jb in range(4):
                nc.tensor.matmul(
                    po3[:, :, 32 * jb : 32 * jb + 32],
                    xt[:, 512 * t + 128 * jb : 512 * t + 128 * jb + 128],
                    b4all[:, 128 * jb : 128 * jb + 128],
                    start=True, stop=True,
                )
            stg = spool.tile([128, 512], F32, tag="ostage")
            nc.vector.tensor_copy(stg[:, 0:256], po[:, 0:256])
            nc.scalar.copy(stg[:, 256:512], po[:, 256:512])
            (nc.sync if t % 2 == 0 else nc.scalar).dma_start(
                out3[:, 4 * t : 4 * t + 4, :],
                stg[:].rearrange("c (r j) -> c r j", j=128),
            )

    nc.compile()
    return nc


_NC_CACHE = None


def _get_nc():
    global _NC_CACHE
    if _NC_CACHE is None:
        _NC_CACHE = build_kernel()
    return _NC_CACHE


def kernel(**inputs) -> np.ndarray:
    X = np.asarray(inputs["X"], np.float32)
    consts = _host_consts(
        {k: np.asarray(v, np.float32) for k, v in inputs.items() if k != "X"}
    )
    in_maps = []
    for core in range(NCORES):
        xs, mrow, emask, xt = _host_shard(X, core)
        m = dict(consts)
        m["x"] = xs
        m["mrow"] = mrow
        m["emask"] = emask
        m["xt"] = xt
        in_maps.append(m)

    nc = _get_nc()
    res = run_bass_kernel_spmd(nc, in_maps, core_ids=list(range(NCORES)))
    out = np.zeros((2, C, 128, 128), np.float32)
    for core in range(NCORES):
        b, ri = divmod(core, 4)
        out[b, :, 32 * ri : 32 * ri + 32, :] = (
            res.results[core]["out"].reshape(C, 32, 128)
        )
    return out


if __name__ == "__main__":
    print("smoke build only")
    build_kernel()
    print("build ok")
